# revision 26
# baseline (speedup 1.0000x reference)
"""Cached multi-head attention, head-sharded (tensor-parallel) over 8 NeuronCores.

Per core: 2 of 16 heads. All matmul operands fp16 (PSUM accumulation fp32).
Flash-style attention in S^T layout (keys on partitions). The softmax
denominator is built on the DVE/GpSimd (prob-tile accumulation) and transposed
to token-partitions with 8 tiny matmuls per query pair, so the PE streams each
probability tile only once (AV) instead of twice (AV + ones-reduction).
New-token K/V stay in SBUF. Projection (b1) and Wo matmuls are interleaved
into the attention windows with even pacing so the PE never idles and its HAM
clock gate stays at 2.4 GHz. Partial Wo outputs (fp16) are summed on the host.
"""
import itertools
import sys
import types

sys.path.insert(0, "/opt/trn_rl_repo")

# Provide antenv.axon_hooks (missing in this image) so trace=True works.
try:
    import antenv.axon_hooks  # noqa: F401
except ImportError:
    try:
        import antenv
        from trn_agent_boot.trn_boot import _ntff_profile_via_ctypes

        _mod = types.ModuleType("antenv.axon_hooks")
        _hook = _ntff_profile_via_ctypes("/opt/axon/libaxon_pjrt.so")
        _mod.get_axon_ntff_profile_hook = lambda: _hook
        _mod.set_axon_ntff_profile_hook = lambda h: None
        sys.modules["antenv.axon_hooks"] = _mod
        antenv.axon_hooks = _mod
    except Exception:
        pass

import numpy as np
import concourse.bass as bass  # noqa: F401
from concourse import bacc
import concourse.mybir as mybir
import concourse.tile as tile
from concourse.bass_utils import run_bass_kernel_spmd

F32 = mybir.dt.float32
F16 = mybir.dt.float16
EXP = mybir.ActivationFunctionType.Exp
COPY_F = mybir.ActivationFunctionType.Copy
MULT = mybir.AluOpType.mult
ADD = mybir.AluOpType.add

P = 128
B = 2
NCORES = 8
HPC = 2              # heads per core
D = 2048             # d_model
DK = 128             # head dim
EC = HPC * DK        # 256 output dims per core
SN = 2048            # new tokens
SP = 2048            # past tokens
DT = D // P          # 16 d-tiles
SCH = 512            # phase-1 s-chunk
NSC = SN // SCH      # 4 chunks per batch
QCH = 512            # q chunk
NQC = SN // QCH      # 4
NPT = SP // P        # 16 past k-tiles
SCALE = float(1.0 / np.sqrt(DK))
PF = 4               # kv-tile DMA prefetch depth

N_PROJ_STEPS = NSC * (2 * HPC * (DT // 4 + 1) + 4 * (DT // 4 + 1)) + 1  # 161
N_ATTN_SLOTS = 2 * ((NPT + 4 * 1 + 4) + (NPT + 4 * 3 + 4)) + 4     # 116

_CACHED_NC = None


def _build():
    nc = bacc.Bacc("TRN2", target_bir_lowering=False, debug=False, num_devices=NCORES)

    xT = nc.dram_tensor("xT", [B, D, SN], F16, kind="ExternalInput")
    wqT = nc.dram_tensor("wqT", [D, EC], F16, kind="ExternalInput")
    wkT = nc.dram_tensor("wkT", [D, EC], F16, kind="ExternalInput")
    wvT = nc.dram_tensor("wvT", [D, EC], F16, kind="ExternalInput")
    woT = nc.dram_tensor("woT", [EC, D], F16, kind="ExternalInput")
    # packed per-k-tile records: [.., kt, 128, 0:128]=K^T tile, [.., 128:256]=V tile
    pkv = nc.dram_tensor("pkv", [B, HPC, NPT, P, 2 * DK], F16, kind="ExternalInput")
    masks = nc.dram_tensor("masks", [P, 4, QCH], F16, kind="ExternalInput")
    ones_d = nc.dram_tensor("ones", [P, 1], F16, kind="ExternalInput")
    out = nc.dram_tensor("out", [B, SN, D], F16, kind="ExternalOutput")

    from contextlib import ExitStack
    with tile.TileContext(nc) as tc, ExitStack() as stack:
        cpool = stack.enter_context(tc.tile_pool(name="const", bufs=1))
        qt_pool = stack.enter_context(tc.tile_pool(name="qt", bufs=2))
        kvn_pool = stack.enter_context(tc.tile_pool(name="kvn", bufs=2))
        ot_pool = stack.enter_context(tc.tile_pool(name="ot", bufs=2))
        recip_pool = stack.enter_context(tc.tile_pool(name="recip", bufs=2))

        t_wq = cpool.tile([P, DT, EC], F16, tag="wq")
        t_wk = cpool.tile([P, DT, EC], F16, tag="wk")
        t_wv = cpool.tile([P, DT, EC], F16, tag="wv")
        t_woT = cpool.tile([P, HPC, D], F16, tag="woT")
        t_masks = cpool.tile([P, 4, QCH], F16, tag="masks")
        t_ones = cpool.tile([P, 1], F16, tag="ones")

        def load_w(t_w, w_d):
            # split loads into 4 pieces to spread across DMA queues
            wr = w_d.rearrange("(t p) e -> p t e", p=P)
            for piece in range(4):
                nc.sync.dma_start(
                    t_w[:, piece * 4:(piece + 1) * 4, :],
                    wr[:, piece * 4:(piece + 1) * 4, :])

        qt = {}
        ktn = {}
        vtn = {}
        ot = {}
        rcp = {}

        def proj_steps(b, xpool, pp1):
            """Q/K/V projections for batch b as a generator of small steps."""
            qt[b] = qt_pool.tile([P, HPC, SN], F16, tag="qt", name=f"qt{b}")
            ktn[b] = kvn_pool.tile([P, HPC, SN], F16, tag="ktn", name=f"ktn{b}")
            vtn[b] = kvn_pool.tile([P, SN // P, EC], F16, tag="vtn", name=f"vtn{b}")
            xr = xT[b].rearrange("(t p) s -> p t s", p=P)
            xts = {}

            def load_x(sc):
                s0 = sc * SCH
                xt = xpool.tile([P, DT, SCH], F16, tag="xt", name=f"xt{b}_{sc}")
                for piece in range(8):
                    nc.sync.dma_start(
                        xt[:, piece * 2:(piece + 1) * 2, :],
                        xr[:, piece * 2:(piece + 1) * 2, s0:s0 + SCH])
                xts[sc] = xt

            load_x(0)
            yield  # pure-prefetch step: x chunk 0 DMA issued, no compute yet
            for sc in range(NSC):
                s0 = sc * SCH
                xt = xts.pop(sc)
                if sc + 1 < NSC:
                    load_x(sc + 1)
                for h in range(HPC):
                    e0 = h * DK
                    psq = pp1.tile([P, SCH], F32, tag="ps1", name="psq")
                    for dt in range(DT):
                        nc.tensor.matmul(
                            psq, t_wq[:, dt, e0:e0 + DK], xt[:, dt, :],
                            start=(dt == 0), stop=(dt == DT - 1))
                        if dt % 4 == 3:
                            yield
                    nc.vector.tensor_copy(qt[b][:, h, s0:s0 + SCH], psq)
                    yield
                    if b == 0 and sc == 0 and h == 0:
                        load_w(t_wk, wkT)
                        load_w(t_wv, wvT)
                        nc.sync.dma_start(t_masks, masks[:, :, :])
                        nc.sync.dma_start(t_ones, ones_d[:, :])
                        wor = woT.rearrange("(h p) d -> p h d", p=P)
                        for hh in range(HPC):
                            for piece in range(2):
                                e0p = piece * (D // 2)
                                nc.sync.dma_start(
                                    t_woT[:, hh, e0p:e0p + D // 2],
                                    wor[:, hh, e0p:e0p + D // 2])
                    psk = pp1.tile([P, SCH], F32, tag="ps1", name="psk")
                    for dt in range(DT):
                        nc.tensor.matmul(
                            psk, t_wk[:, dt, e0:e0 + DK], xt[:, dt, :],
                            start=(dt == 0), stop=(dt == DT - 1))
                        if dt % 4 == 3:
                            yield
                    nc.vector.tensor_copy(ktn[b][:, h, s0:s0 + SCH], psk)
                    yield
                for sub in range(SCH // P):
                    psv = pp1.tile([P, SCH], F32, tag="ps1", name="psv")
                    for dt in range(DT):
                        nc.tensor.matmul(
                            psv[:, 0:EC], xt[:, dt, sub * P:(sub + 1) * P],
                            t_wv[:, dt, :],
                            start=(dt == 0), stop=(dt == DT - 1))
                        if dt % 4 == 3:
                            yield
                    nc.vector.tensor_copy(vtn[b][:, sc * 4 + sub, :], psv[:, 0:EC])
                    yield

        def emit_attn(b, kvpool, ptpool, accpool, pp2, ppo, ilv):
            """Attention for both heads of batch b (flash-style, pr-major)."""
            ot[b] = ot_pool.tile([P, HPC, SN], F16, tag="ot", name=f"ot{b}")
            rcp[b] = recip_pool.tile([P, HPC, 16], F32, tag="rcp", name=f"rcp{b}")
            for pr in range(NQC // 2):
                for h in range(HPC):
                    qA, qB = 2 * pr, 2 * pr + 1
                    q0 = qA * QCH
                    nkts = NPT + 4 * qB + 4
                    lim_a = NPT + 4 * qA + 4
                    po = ppo.tile([P, 2 * QCH], F32, tag="po", name="po")
                    acc = accpool.tile([P, 2 * QCH], F16, tag="acc", name="acc")
                    state = {"fa": True, "fb": True}
                    pend = []
                    kv_sb = {}

                    def a_valid(kt):
                        return kt < lim_a

                    def load_ktile(kt):
                        if kt < NPT:
                            kv_t = kvpool.tile([P, 2 * DK], F16, tag="kvt",
                                               name="kvt")
                            nc.sync.dma_start(kv_t, pkv[b, h, kt])
                            kv_sb[kt] = (kv_t[:, 0:DK], kv_t[:, DK:2 * DK])
                        else:
                            t = kt - NPT
                            kv_sb[kt] = (
                                ktn[b][:, h, t * P:(t + 1) * P],
                                vtn[b][:, t, h * DK:(h + 1) * DK])

                    def drain(last):
                        kt, ps_s, pt = pend.pop(0)
                        k_t, v_t = kv_sb.pop(kt)
                        av = a_valid(kt)
                        lo = 0 if av else QCH
                        nc.scalar.activation(
                            pt[:, lo:], ps_s[:, lo:], EXP, scale=SCALE)
                        ktn_i = kt - NPT
                        if 4 * qA <= ktn_i < 4 * qA + 4:
                            nc.vector.tensor_mul(
                                pt[:, 0:QCH], pt[:, 0:QCH],
                                t_masks[:, ktn_i - 4 * qA, :])
                        if 4 * qB <= ktn_i < 4 * qB + 4:
                            nc.vector.tensor_mul(
                                pt[:, QCH:], pt[:, QCH:],
                                t_masks[:, ktn_i - 4 * qB, :])
                        if kt == 0:
                            nc.vector.tensor_copy(acc, pt)
                        else:
                            nc.vector.tensor_add(
                                acc[:, lo:], acc[:, lo:], pt[:, lo:])
                        lastA = av and (last or kt == lim_a - 1)
                        if av:
                            nc.tensor.matmul(
                                po[:, 0:QCH], v_t, pt[:, 0:QCH],
                                start=state["fa"], stop=lastA)
                            state["fa"] = False
                        nc.tensor.matmul(
                            po[:, QCH:], v_t, pt[:, QCH:],
                            start=state["fb"], stop=last)
                        state["fb"] = False

                    for i in range(nkts):
                        if i == 0:
                            for j in range(min(PF, nkts)):
                                load_ktile(j)
                        elif i + PF - 1 < nkts:
                            load_ktile(i + PF - 1)
                        kt = i
                        k_t, _ = kv_sb[kt]
                        ps_s = pp2.tile([P, 2 * QCH], F32, tag="ps_s",
                                        name="ps_s")
                        pt = ptpool.tile([P, 2 * QCH], F16, tag="pt", name="pt")
                        if a_valid(kt):
                            nc.tensor.matmul(
                                ps_s[:, 0:QCH], k_t,
                                qt[b][:, h, q0:q0 + QCH],
                                start=True, stop=True)
                        nc.tensor.matmul(
                            ps_s[:, QCH:], k_t,
                            qt[b][:, h, q0 + QCH:q0 + 2 * QCH],
                            start=True, stop=True)
                        pend.append((kt, ps_s, pt))
                        # interleave BEFORE the drain: the filler matmuls sit
                        # between scores(i) and AV(i-1) in the PE queue, giving
                        # exp(i-1) time to finish so AV never head-of-line
                        # blocks the PE
                        ilv()
                        if len(pend) > 1:
                            drain(False)
                    drain(True)

                    # PE filler first: the denominator matmuls below wait on
                    # the DVE's accumulator backlog
                    ilv()
                    # denominators: reduce+transpose acc via 8 N=1 matmuls
                    pst = pp2.tile([P, 2 * QCH], F32, tag="ps_s", name="pst")
                    for c in range(8):
                        nc.tensor.matmul(
                            pst[:, c:c + 1], acc[:, c * P:(c + 1) * P],
                            t_ones, start=True, stop=True)
                    nc.vector.reciprocal(
                        rcp[b][:, h, pr * 8:(pr + 1) * 8], pst[:, 0:8])
                    # po can exceed fp16 max; scale by 1/64 here and bake the
                    # compensating *64 into the denominators (ones = 1/64)
                    nc.scalar.activation(
                        ot[b][:, h, q0:q0 + 2 * QCH], po, COPY_F,
                        scale=1.0 / 64)

        def oproj_steps(b, sts, opool, ppso, alt0=0, tmp_act_mod=2):
            """Wo projection tiles for token blocks `sts` of batch b."""
            for idx, st in enumerate(sts):
                r0 = st * P
                for ec in range(D // QCH):
                    e0 = ec * QCH
                    pso0 = ppso.tile([P, SCH], F32, tag="ps1", name="pso0")
                    nc.tensor.matmul(
                        pso0, ot[b][:, 0, r0:r0 + P],
                        t_woT[:, 0, e0:e0 + QCH], start=True, stop=True)
                    pso1 = ppso.tile([P, SCH], F32, tag="ps1", name="pso1")
                    nc.tensor.matmul(
                        pso1, ot[b][:, 1, r0:r0 + P],
                        t_woT[:, 1, e0:e0 + QCH], start=True, stop=True)
                    tmp = opool.tile([P, QCH], F16, tag="tmp", name="tmp")
                    if (idx * 4 + ec + alt0) % tmp_act_mod == 0:
                        nc.scalar.activation(
                            tmp, pso0, COPY_F, scale=rcp[b][:, 0, st:st + 1])
                    else:
                        nc.vector.tensor_scalar_mul(
                            tmp, pso0, rcp[b][:, 0, st:st + 1])
                    outt = opool.tile([P, QCH], F16, tag="outt", name="outt")
                    nc.vector.scalar_tensor_tensor(
                        outt, pso1, rcp[b][:, 1, st:st + 1], tmp,
                        op0=MULT, op1=ADD)
                    nc.sync.dma_start(out[b, r0:r0 + P, e0:e0 + QCH], outt)
                    yield

        def oproj_wide(b, sts, opool, pp2, ppo):
            """Wide (2-ec) Wo projection tiles for the dense tail window.

            Runs after attention is fully drained, so it reuses the attention
            PSUM pools (ps_s/po tags) — avoiding a pool transition that would
            idle the PE and drop the HAM clock gate back to 1.2 GHz.
            """
            for st in sts:
                r0 = st * P
                for ec2 in range(D // QCH // 2):
                    e0 = ec2 * 2 * QCH
                    pso0 = pp2.tile([P, 2 * QCH], F32, tag="ps_s", name="pso0w")
                    pso1 = ppo.tile([P, 2 * QCH], F32, tag="po", name="pso1w")
                    for half in range(2):
                        hs = half * QCH
                        nc.tensor.matmul(
                            pso0[:, hs:hs + QCH], ot[b][:, 0, r0:r0 + P],
                            t_woT[:, 0, e0 + hs:e0 + hs + QCH],
                            start=True, stop=True)
                        nc.tensor.matmul(
                            pso1[:, hs:hs + QCH], ot[b][:, 1, r0:r0 + P],
                            t_woT[:, 1, e0 + hs:e0 + hs + QCH],
                            start=True, stop=True)
                    tmp = opool.tile([P, 2 * QCH], F16, tag="tmpw", name="tmpw")
                    nc.scalar.activation(
                        tmp, pso0, COPY_F, scale=rcp[b][:, 0, st:st + 1])
                    outt = opool.tile([P, 2 * QCH], F16, tag="outtw",
                                      name="outtw")
                    nc.vector.scalar_tensor_tensor(
                        outt, pso1, rcp[b][:, 1, st:st + 1], tmp,
                        op0=MULT, op1=ADD)
                    nc.sync.dma_start(out[b, r0:r0 + P, e0:e0 + 2 * QCH], outt)
                    yield

        def make_ilv(parts, slots):
            """Evenly pace (generator, count, min_slot) parts across `slots`.

            A part is never advanced before its min_slot'th interleave call —
            emission-order gating for work that reads tiles written by
            attention pairs emitted earlier in the same window.
            """
            total = sum(c for _, c, _ in parts)
            st = {"slot": 0, "idx": 0, "consumed": 0}

            def ilv():
                st["slot"] += 1
                # linear target: consume evenly across slots (ceil pacing
                # front-loads and leaves late slots with no PE filler)
                target = min(total, (total * st["slot"] + slots - 1) // slots)
                while st["consumed"] < target:
                    advanced = False
                    while st["idx"] < len(parts):
                        g, _c, ms = parts[st["idx"]]
                        if st["slot"] < ms:
                            return
                        if next(g, None) is None:
                            st["idx"] += 1
                            continue
                        st["consumed"] += 1
                        advanced = True
                        break
                    if not advanced:
                        return

            def drain_rest():
                for g, _c, _ms in parts:
                    for _ in g:
                        pass
            return ilv, drain_rest

        # ---------------- schedule ----------------
        load_w(t_wq, wqT)
        with tc.tile_pool(name="kv", bufs=8) as kvpool, \
             tc.tile_pool(name="pt", bufs=4) as ptpool, \
             tc.tile_pool(name="acc", bufs=2) as accpool, \
             tc.tile_pool(name="ps2", bufs=2, space="PSUM") as pp2, \
             tc.tile_pool(name="po", bufs=1, space="PSUM") as ppo:
            with tc.tile_pool(name="ps1", bufs=2, space="PSUM") as pp1, \
                 tc.tile_pool(name="os", bufs=3) as opool:
                with tc.tile_pool(name="xt", bufs=2) as xpool:
                    g0 = proj_steps(0, xpool, pp1)
                    g1 = proj_steps(1, xpool, pp1)
                    # proj b0 runs fully dense (interleaving a split chain
                    # into attention startup corrupts its V tiles); prefetch
                    # b1's first x chunk near the end so g1's first compute
                    # steps are DMA-ready when attention interleaving begins
                    for i in range(N_PROJ_STEPS):
                        next(g0)
                        if i == N_PROJ_STEPS - 20:
                            next(g1)
                    # w2's Activation engine is pegged by exp: put gw0a's
                    # normalization tmp ops on the DVE (which has slack here)
                    gw0a = oproj_steps(0, range(0, 8), opool, pp1,
                                       tmp_act_mod=4)
                    ilv2, drain2 = make_ilv(
                        [(g1, N_PROJ_STEPS - 1, 0), (gw0a, 32, 52)],
                        N_ATTN_SLOTS)
                    emit_attn(0, kvpool, ptpool, accpool, pp2, ppo, ilv2)
                    drain2()
                gw0b = oproj_steps(0, range(8, 16), opool, pp1, alt0=1)
                gw1a = oproj_steps(1, range(0, 8), opool, pp1)
                ilv3, drain3 = make_ilv(
                    [(gw0b, 32, 0), (gw1a, 32, 52)], N_ATTN_SLOTS)
                emit_attn(1, kvpool, ptpool, accpool, pp2, ppo, ilv3)
                drain3()
                for _ in oproj_wide(1, range(8, 16), opool, pp2, ppo):
                    pass

    nc.compile()
    return nc


def _get_nc():
    global _CACHED_NC
    if _CACHED_NC is None:
        _CACHED_NC = _build()
    return _CACHED_NC


def pack_kv(pk, pvv):
    # [B, HPC, S, DK] -> [B, HPC, S//P, P, 2*DK]: [.., 0:DK]=K^T tile, [.., DK:]=V tile
    b, hpc, s, dk = pk.shape
    kt = pk.reshape(b, hpc, s // P, P, dk).transpose(0, 1, 2, 4, 3)
    vt = pvv.reshape(b, hpc, s // P, P, dk)
    return np.ascontiguousarray(
        np.concatenate([kt, vt], axis=4).astype(np.float16))


def _prep_inputs(x, past_key, past_value, Wq, Wk, Wv, Wo):
    x = np.asarray(x, np.float32)
    past_key = np.asarray(past_key, np.float32)
    past_value = np.asarray(past_value, np.float32)
    Wq = np.asarray(Wq, np.float32)
    Wk = np.asarray(Wk, np.float32)
    Wv = np.asarray(Wv, np.float32)
    Wo = np.asarray(Wo, np.float32)

    xT = np.ascontiguousarray(x.transpose(0, 2, 1).astype(np.float16))
    i = np.arange(P)[:, None]
    j = np.arange(QCH)[None, :]
    m = np.stack([(j >= i + o * P) for o in range(4)], axis=1).astype(np.float16)
    m = np.ascontiguousarray(m)  # [P, 4, QCH]
    ones = np.full((P, 1), 1.0 / 64, np.float16)

    in_maps = []
    for c in range(NCORES):
        e0 = c * EC
        hs = slice(c * HPC, (c + 1) * HPC)
        in_maps.append({
            "xT": xT,
            "wqT": np.ascontiguousarray(Wq[e0:e0 + EC, :].T.astype(np.float16)),
            "wkT": np.ascontiguousarray(Wk[e0:e0 + EC, :].T.astype(np.float16)),
            "wvT": np.ascontiguousarray(Wv[e0:e0 + EC, :].T.astype(np.float16)),
            "woT": np.ascontiguousarray(Wo[:, e0:e0 + EC].T.astype(np.float16)),
            "pkv": pack_kv(past_key[:, hs], past_value[:, hs]),
            "masks": m,
            "ones": ones,
        })
    return in_maps


def _run(inputs, trace=False):
    nc = _get_nc()
    in_maps = _prep_inputs(**inputs)
    res = run_bass_kernel_spmd(nc, in_maps, core_ids=list(range(NCORES)), trace=trace)
    total = res.results[0]["out"].astype(np.float32)
    for c in range(1, NCORES):
        total += res.results[c]["out"].astype(np.float32)
    return total, res


def kernel(x, past_key, past_value, Wq, Wk, Wv, Wo):
    total, _ = _run(dict(x=x, past_key=past_key, past_value=past_value,
                         Wq=Wq, Wk=Wk, Wv=Wv, Wo=Wo))
    return total


# revision 28
# speedup vs baseline: 1.0183x; 1.0183x over previous
"""Cached multi-head attention, head-sharded (tensor-parallel) over 8 NeuronCores.

Per core: 2 of 16 heads. All matmul operands fp16 (PSUM accumulation fp32).
Flash-style attention in S^T layout (keys on partitions). The softmax
denominator is built on the DVE/GpSimd (prob-tile accumulation) and transposed
to token-partitions with 8 tiny matmuls per query pair, so the PE streams each
probability tile only once (AV) instead of twice (AV + ones-reduction).
New-token K/V stay in SBUF. Projection (b1) and Wo matmuls are interleaved
into the attention windows with even pacing so the PE never idles and its HAM
clock gate stays at 2.4 GHz. Partial Wo outputs (fp16) are summed on the host.
"""
import itertools
import sys
import types

sys.path.insert(0, "/opt/trn_rl_repo")

# Provide antenv.axon_hooks (missing in this image) so trace=True works.
try:
    import antenv.axon_hooks  # noqa: F401
except ImportError:
    try:
        import antenv
        from trn_agent_boot.trn_boot import _ntff_profile_via_ctypes

        _mod = types.ModuleType("antenv.axon_hooks")
        _hook = _ntff_profile_via_ctypes("/opt/axon/libaxon_pjrt.so")
        _mod.get_axon_ntff_profile_hook = lambda: _hook
        _mod.set_axon_ntff_profile_hook = lambda h: None
        sys.modules["antenv.axon_hooks"] = _mod
        antenv.axon_hooks = _mod
    except Exception:
        pass

import numpy as np
import concourse.bass as bass  # noqa: F401
from concourse import bacc
import concourse.mybir as mybir
import concourse.tile as tile
from concourse.bass_utils import run_bass_kernel_spmd

F32 = mybir.dt.float32
F16 = mybir.dt.float16
EXP = mybir.ActivationFunctionType.Exp
COPY_F = mybir.ActivationFunctionType.Copy
MULT = mybir.AluOpType.mult
ADD = mybir.AluOpType.add

P = 128
B = 2
NCORES = 8
HPC = 2              # heads per core
D = 2048             # d_model
DK = 128             # head dim
EC = HPC * DK        # 256 output dims per core
SN = 2048            # new tokens
SP = 2048            # past tokens
DT = D // P          # 16 d-tiles
SCH = 512            # phase-1 s-chunk
NSC = SN // SCH      # 4 chunks per batch
QCH = 512            # q chunk
NQC = SN // QCH      # 4
NPT = SP // P        # 16 past k-tiles
SCALE = float(1.0 / np.sqrt(DK))
PF = 4               # kv-tile DMA prefetch depth

N_PROJ_STEPS = NSC * (2 * HPC * (DT // 4 + 1) + 4 * (DT // 4 + 1)) + 1  # 161
N_ATTN_SLOTS = 2 * ((NPT + 4 * 1 + 4) + (NPT + 4 * 3 + 4)) + 4     # 116

_CACHED_NC = None


def _build():
    nc = bacc.Bacc("TRN2", target_bir_lowering=False, debug=False, num_devices=NCORES)

    xT = nc.dram_tensor("xT", [B, D, SN], F16, kind="ExternalInput")
    wqT = nc.dram_tensor("wqT", [D, EC], F16, kind="ExternalInput")
    wkT = nc.dram_tensor("wkT", [D, EC], F16, kind="ExternalInput")
    wvT = nc.dram_tensor("wvT", [D, EC], F16, kind="ExternalInput")
    woT = nc.dram_tensor("woT", [EC, D], F16, kind="ExternalInput")
    # packed per-k-tile records: [.., kt, 128, 0:128]=K^T tile, [.., 128:256]=V tile
    pkv = nc.dram_tensor("pkv", [B, HPC, NPT, P, 2 * DK], F16, kind="ExternalInput")
    masks = nc.dram_tensor("masks", [P, 4, QCH], F16, kind="ExternalInput")
    ones_d = nc.dram_tensor("ones", [P, 1], F16, kind="ExternalInput")
    out = nc.dram_tensor("out", [B, SN, D], F16, kind="ExternalOutput")

    from contextlib import ExitStack
    with tile.TileContext(nc) as tc, ExitStack() as stack:
        cpool = stack.enter_context(tc.tile_pool(name="const", bufs=1))
        qt_pool = stack.enter_context(tc.tile_pool(name="qt", bufs=2))
        kvn_pool = stack.enter_context(tc.tile_pool(name="kvn", bufs=2))
        ot_pool = stack.enter_context(tc.tile_pool(name="ot", bufs=2))
        recip_pool = stack.enter_context(tc.tile_pool(name="recip", bufs=2))

        t_wq = cpool.tile([P, DT, EC], F16, tag="wq")
        t_wk = cpool.tile([P, DT, EC], F16, tag="wk")
        t_wv = cpool.tile([P, DT, EC], F16, tag="wv")
        t_woT = cpool.tile([P, HPC, D], F16, tag="woT")
        t_masks = cpool.tile([P, 4, QCH], F16, tag="masks")
        t_ones = cpool.tile([P, 1], F16, tag="ones")

        def load_w(t_w, w_d):
            # split loads into 4 pieces to spread across DMA queues
            wr = w_d.rearrange("(t p) e -> p t e", p=P)
            for piece in range(4):
                nc.sync.dma_start(
                    t_w[:, piece * 4:(piece + 1) * 4, :],
                    wr[:, piece * 4:(piece + 1) * 4, :])

        qt = {}
        ktn = {}
        vtn = {}
        ot = {}
        rcp = {}

        def proj_steps(b, xpool, pp1):
            """Q/K/V projections for batch b as a generator of small steps."""
            qt[b] = qt_pool.tile([P, HPC, SN], F16, tag="qt", name=f"qt{b}")
            ktn[b] = kvn_pool.tile([P, HPC, SN], F16, tag="ktn", name=f"ktn{b}")
            vtn[b] = kvn_pool.tile([P, SN // P, EC], F16, tag="vtn", name=f"vtn{b}")
            xr = xT[b].rearrange("(t p) s -> p t s", p=P)
            xts = {}

            def load_x(sc):
                s0 = sc * SCH
                xt = xpool.tile([P, DT, SCH], F16, tag="xt", name=f"xt{b}_{sc}")
                for piece in range(8):
                    nc.sync.dma_start(
                        xt[:, piece * 2:(piece + 1) * 2, :],
                        xr[:, piece * 2:(piece + 1) * 2, s0:s0 + SCH])
                xts[sc] = xt

            load_x(0)
            yield  # pure-prefetch step: x chunk 0 DMA issued, no compute yet
            for sc in range(NSC):
                s0 = sc * SCH
                xt = xts.pop(sc)
                if sc + 1 < NSC:
                    load_x(sc + 1)
                for h in range(HPC):
                    e0 = h * DK
                    psq = pp1.tile([P, SCH], F32, tag="ps1", name="psq")
                    for dt in range(DT):
                        nc.tensor.matmul(
                            psq, t_wq[:, dt, e0:e0 + DK], xt[:, dt, :],
                            start=(dt == 0), stop=(dt == DT - 1))
                        if dt % 4 == 3:
                            yield
                    nc.vector.tensor_copy(qt[b][:, h, s0:s0 + SCH], psq)
                    yield
                    if b == 0 and sc == 0 and h == 0:
                        load_w(t_wk, wkT)
                        load_w(t_wv, wvT)
                        nc.sync.dma_start(t_masks, masks[:, :, :])
                        nc.sync.dma_start(t_ones, ones_d[:, :])
                        wor = woT.rearrange("(h p) d -> p h d", p=P)
                        for hh in range(HPC):
                            for piece in range(2):
                                e0p = piece * (D // 2)
                                nc.sync.dma_start(
                                    t_woT[:, hh, e0p:e0p + D // 2],
                                    wor[:, hh, e0p:e0p + D // 2])
                    psk = pp1.tile([P, SCH], F32, tag="ps1", name="psk")
                    for dt in range(DT):
                        nc.tensor.matmul(
                            psk, t_wk[:, dt, e0:e0 + DK], xt[:, dt, :],
                            start=(dt == 0), stop=(dt == DT - 1))
                        if dt % 4 == 3:
                            yield
                    nc.vector.tensor_copy(ktn[b][:, h, s0:s0 + SCH], psk)
                    yield
                for sub in range(SCH // P):
                    psv = pp1.tile([P, SCH], F32, tag="ps1", name="psv")
                    for dt in range(DT):
                        nc.tensor.matmul(
                            psv[:, 0:EC], xt[:, dt, sub * P:(sub + 1) * P],
                            t_wv[:, dt, :],
                            start=(dt == 0), stop=(dt == DT - 1))
                        if dt % 4 == 3:
                            yield
                    nc.vector.tensor_copy(vtn[b][:, sc * 4 + sub, :], psv[:, 0:EC])
                    yield

        def emit_attn(b, kvpool, ptpool, accpool, pp2, ppo, ilv):
            """Attention for both heads of batch b (flash-style, pr-major)."""
            ot[b] = ot_pool.tile([P, HPC, SN], F16, tag="ot", name=f"ot{b}")
            rcp[b] = recip_pool.tile([P, HPC, 16], F32, tag="rcp", name=f"rcp{b}")
            for pr in range(NQC // 2):
                for h in range(HPC):
                    qA, qB = 2 * pr, 2 * pr + 1
                    q0 = qA * QCH
                    nkts = NPT + 4 * qB + 4
                    lim_a = NPT + 4 * qA + 4
                    po = ppo.tile([P, 2 * QCH], F32, tag="po", name="po")
                    acc = accpool.tile([P, 2 * QCH], F16, tag="acc", name="acc")
                    state = {"fa": True, "fb": True}
                    pend = []
                    kv_sb = {}

                    def a_valid(kt):
                        return kt < lim_a

                    def load_ktile(kt):
                        if kt < NPT:
                            kv_t = kvpool.tile([P, 2 * DK], F16, tag="kvt",
                                               name="kvt")
                            nc.sync.dma_start(kv_t, pkv[b, h, kt])
                            kv_sb[kt] = (kv_t[:, 0:DK], kv_t[:, DK:2 * DK])
                        else:
                            t = kt - NPT
                            kv_sb[kt] = (
                                ktn[b][:, h, t * P:(t + 1) * P],
                                vtn[b][:, t, h * DK:(h + 1) * DK])

                    def drain(last):
                        kt, ps_s, pt = pend.pop(0)
                        k_t, v_t = kv_sb.pop(kt)
                        av = a_valid(kt)
                        lo = 0 if av else QCH
                        nc.scalar.activation(
                            pt[:, lo:], ps_s[:, lo:], EXP, scale=SCALE)
                        ktn_i = kt - NPT
                        if 4 * qA <= ktn_i < 4 * qA + 4:
                            nc.vector.tensor_mul(
                                pt[:, 0:QCH], pt[:, 0:QCH],
                                t_masks[:, ktn_i - 4 * qA, :])
                        if 4 * qB <= ktn_i < 4 * qB + 4:
                            nc.vector.tensor_mul(
                                pt[:, QCH:], pt[:, QCH:],
                                t_masks[:, ktn_i - 4 * qB, :])
                        if kt == 0:
                            nc.vector.tensor_copy(acc, pt)
                        else:
                            nc.vector.tensor_add(
                                acc[:, lo:], acc[:, lo:], pt[:, lo:])
                        lastA = av and (last or kt == lim_a - 1)
                        if av:
                            nc.tensor.matmul(
                                po[:, 0:QCH], v_t, pt[:, 0:QCH],
                                start=state["fa"], stop=lastA)
                            state["fa"] = False
                        nc.tensor.matmul(
                            po[:, QCH:], v_t, pt[:, QCH:],
                            start=state["fb"], stop=last)
                        state["fb"] = False

                    for i in range(nkts):
                        if i == 0:
                            for j in range(min(PF, nkts)):
                                load_ktile(j)
                        elif i + PF - 1 < nkts:
                            load_ktile(i + PF - 1)
                        kt = i
                        k_t, _ = kv_sb[kt]
                        ps_s = pp2.tile([P, 2 * QCH], F32, tag="ps_s",
                                        name="ps_s")
                        pt = ptpool.tile([P, 2 * QCH], F16, tag="pt", name="pt")
                        if a_valid(kt):
                            nc.tensor.matmul(
                                ps_s[:, 0:QCH], k_t,
                                qt[b][:, h, q0:q0 + QCH],
                                start=True, stop=True)
                        nc.tensor.matmul(
                            ps_s[:, QCH:], k_t,
                            qt[b][:, h, q0 + QCH:q0 + 2 * QCH],
                            start=True, stop=True)
                        pend.append((kt, ps_s, pt))
                        # interleave BEFORE the drain: the filler matmuls sit
                        # between scores(i) and AV(i-1) in the PE queue, giving
                        # exp(i-1) time to finish so AV never head-of-line
                        # blocks the PE
                        ilv()
                        if len(pend) > 1:
                            drain(False)
                    drain(True)

                    # denominators: reduce+transpose acc via 8 N=1 matmuls
                    pst = pp2.tile([P, 2 * QCH], F32, tag="ps_s", name="pst")
                    for c in range(8):
                        nc.tensor.matmul(
                            pst[:, c:c + 1], acc[:, c * P:(c + 1) * P],
                            t_ones, start=True, stop=True)
                    nc.vector.reciprocal(
                        rcp[b][:, h, pr * 8:(pr + 1) * 8], pst[:, 0:8])
                    # po can exceed fp16 max; scale by 1/64 here and bake the
                    # compensating *64 into the denominators (ones = 1/64)
                    nc.scalar.activation(
                        ot[b][:, h, q0:q0 + 2 * QCH], po, COPY_F,
                        scale=1.0 / 64)
                    ilv()

        def oproj_steps(b, sts, opool, ppso, alt0=0, tmp_act_mod=2):
            """Wo projection tiles for token blocks `sts` of batch b."""
            for idx, st in enumerate(sts):
                r0 = st * P
                for ec in range(D // QCH):
                    e0 = ec * QCH
                    pso0 = ppso.tile([P, SCH], F32, tag="ps1", name="pso0")
                    nc.tensor.matmul(
                        pso0, ot[b][:, 0, r0:r0 + P],
                        t_woT[:, 0, e0:e0 + QCH], start=True, stop=True)
                    pso1 = ppso.tile([P, SCH], F32, tag="ps1", name="pso1")
                    nc.tensor.matmul(
                        pso1, ot[b][:, 1, r0:r0 + P],
                        t_woT[:, 1, e0:e0 + QCH], start=True, stop=True)
                    tmp = opool.tile([P, QCH], F16, tag="tmp", name="tmp")
                    if (idx * 4 + ec + alt0) % tmp_act_mod == 0:
                        nc.scalar.activation(
                            tmp, pso0, COPY_F, scale=rcp[b][:, 0, st:st + 1])
                    else:
                        nc.vector.tensor_scalar_mul(
                            tmp, pso0, rcp[b][:, 0, st:st + 1])
                    outt = opool.tile([P, QCH], F16, tag="outt", name="outt")
                    nc.vector.scalar_tensor_tensor(
                        outt, pso1, rcp[b][:, 1, st:st + 1], tmp,
                        op0=MULT, op1=ADD)
                    nc.sync.dma_start(out[b, r0:r0 + P, e0:e0 + QCH], outt)
                    yield

        def oproj_wide(b, sts, opool, pp2, ppo):
            """Wide (2-ec) Wo projection tiles for the dense tail window.

            Runs after attention is fully drained, so it reuses the attention
            PSUM pools (ps_s/po tags) — avoiding a pool transition that would
            idle the PE and drop the HAM clock gate back to 1.2 GHz.
            """
            for st in sts:
                r0 = st * P
                for ec2 in range(D // QCH // 2):
                    e0 = ec2 * 2 * QCH
                    pso0 = pp2.tile([P, 2 * QCH], F32, tag="ps_s", name="pso0w")
                    pso1 = ppo.tile([P, 2 * QCH], F32, tag="po", name="pso1w")
                    for half in range(2):
                        hs = half * QCH
                        nc.tensor.matmul(
                            pso0[:, hs:hs + QCH], ot[b][:, 0, r0:r0 + P],
                            t_woT[:, 0, e0 + hs:e0 + hs + QCH],
                            start=True, stop=True)
                        nc.tensor.matmul(
                            pso1[:, hs:hs + QCH], ot[b][:, 1, r0:r0 + P],
                            t_woT[:, 1, e0 + hs:e0 + hs + QCH],
                            start=True, stop=True)
                    tmp = opool.tile([P, 2 * QCH], F16, tag="tmpw", name="tmpw")
                    nc.scalar.activation(
                        tmp, pso0, COPY_F, scale=rcp[b][:, 0, st:st + 1])
                    outt = opool.tile([P, 2 * QCH], F16, tag="outtw",
                                      name="outtw")
                    nc.vector.scalar_tensor_tensor(
                        outt, pso1, rcp[b][:, 1, st:st + 1], tmp,
                        op0=MULT, op1=ADD)
                    nc.sync.dma_start(out[b, r0:r0 + P, e0:e0 + 2 * QCH], outt)
                    yield

        def make_ilv(parts, slots):
            """Evenly pace (generator, count, min_slot) parts across `slots`.

            A part is never advanced before its min_slot'th interleave call —
            emission-order gating for work that reads tiles written by
            attention pairs emitted earlier in the same window.
            """
            total = sum(c for _, c, _ in parts)
            st = {"slot": 0, "idx": 0, "consumed": 0}

            def ilv():
                st["slot"] += 1
                # linear target: consume evenly across slots (ceil pacing
                # front-loads and leaves late slots with no PE filler)
                target = min(total, (total * st["slot"] + slots - 1) // slots)
                while st["consumed"] < target:
                    advanced = False
                    while st["idx"] < len(parts):
                        g, _c, ms = parts[st["idx"]]
                        if st["slot"] < ms:
                            return
                        if next(g, None) is None:
                            st["idx"] += 1
                            continue
                        st["consumed"] += 1
                        advanced = True
                        break
                    if not advanced:
                        return

            def drain_rest():
                for g, _c, _ms in parts:
                    for _ in g:
                        pass
            return ilv, drain_rest

        # ---------------- schedule ----------------
        load_w(t_wq, wqT)
        with tc.tile_pool(name="kv", bufs=8) as kvpool, \
             tc.tile_pool(name="pt", bufs=4) as ptpool, \
             tc.tile_pool(name="acc", bufs=2) as accpool, \
             tc.tile_pool(name="ps2", bufs=2, space="PSUM") as pp2, \
             tc.tile_pool(name="po", bufs=1, space="PSUM") as ppo:
            with tc.tile_pool(name="ps1", bufs=2, space="PSUM") as pp1, \
                 tc.tile_pool(name="os", bufs=3) as opool:
                with tc.tile_pool(name="xt", bufs=2) as xpool:
                    g0 = proj_steps(0, xpool, pp1)
                    g1 = proj_steps(1, xpool, pp1)
                    # proj b0 runs fully dense (interleaving a split chain
                    # into attention startup corrupts its V tiles); prefetch
                    # b1's first x chunk near the end so g1's first compute
                    # steps are DMA-ready when attention interleaving begins
                    for i in range(N_PROJ_STEPS):
                        next(g0)
                        if i == N_PROJ_STEPS - 20:
                            next(g1)
                    gw0a = oproj_steps(0, range(0, 8), opool, pp1)
                    ilv2, drain2 = make_ilv(
                        [(g1, N_PROJ_STEPS - 1, 0), (gw0a, 32, 52)],
                        N_ATTN_SLOTS)
                    emit_attn(0, kvpool, ptpool, accpool, pp2, ppo, ilv2)
                    drain2()
                gw0b = oproj_steps(0, range(8, 16), opool, pp1, alt0=1)
                gw1a = oproj_steps(1, range(0, 8), opool, pp1)
                ilv3, drain3 = make_ilv(
                    [(gw0b, 32, 0), (gw1a, 32, 52)], N_ATTN_SLOTS)
                emit_attn(1, kvpool, ptpool, accpool, pp2, ppo, ilv3)
                drain3()
                for _ in oproj_wide(1, range(8, 16), opool, pp2, ppo):
                    pass

    nc.compile()
    return nc


def _get_nc():
    global _CACHED_NC
    if _CACHED_NC is None:
        _CACHED_NC = _build()
    return _CACHED_NC


def pack_kv(pk, pvv):
    # [B, HPC, S, DK] -> [B, HPC, S//P, P, 2*DK]: [.., 0:DK]=K^T tile, [.., DK:]=V tile
    b, hpc, s, dk = pk.shape
    kt = pk.reshape(b, hpc, s // P, P, dk).transpose(0, 1, 2, 4, 3)
    vt = pvv.reshape(b, hpc, s // P, P, dk)
    return np.ascontiguousarray(
        np.concatenate([kt, vt], axis=4).astype(np.float16))


def _prep_inputs(x, past_key, past_value, Wq, Wk, Wv, Wo):
    x = np.asarray(x, np.float32)
    past_key = np.asarray(past_key, np.float32)
    past_value = np.asarray(past_value, np.float32)
    Wq = np.asarray(Wq, np.float32)
    Wk = np.asarray(Wk, np.float32)
    Wv = np.asarray(Wv, np.float32)
    Wo = np.asarray(Wo, np.float32)

    xT = np.ascontiguousarray(x.transpose(0, 2, 1).astype(np.float16))
    i = np.arange(P)[:, None]
    j = np.arange(QCH)[None, :]
    m = np.stack([(j >= i + o * P) for o in range(4)], axis=1).astype(np.float16)
    m = np.ascontiguousarray(m)  # [P, 4, QCH]
    ones = np.full((P, 1), 1.0 / 64, np.float16)

    in_maps = []
    for c in range(NCORES):
        e0 = c * EC
        hs = slice(c * HPC, (c + 1) * HPC)
        in_maps.append({
            "xT": xT,
            "wqT": np.ascontiguousarray(Wq[e0:e0 + EC, :].T.astype(np.float16)),
            "wkT": np.ascontiguousarray(Wk[e0:e0 + EC, :].T.astype(np.float16)),
            "wvT": np.ascontiguousarray(Wv[e0:e0 + EC, :].T.astype(np.float16)),
            "woT": np.ascontiguousarray(Wo[:, e0:e0 + EC].T.astype(np.float16)),
            "pkv": pack_kv(past_key[:, hs], past_value[:, hs]),
            "masks": m,
            "ones": ones,
        })
    return in_maps


def _run(inputs, trace=False):
    nc = _get_nc()
    in_maps = _prep_inputs(**inputs)
    res = run_bass_kernel_spmd(nc, in_maps, core_ids=list(range(NCORES)), trace=trace)
    total = res.results[0]["out"].astype(np.float32)
    for c in range(1, NCORES):
        total += res.results[c]["out"].astype(np.float32)
    return total, res


def kernel(x, past_key, past_value, Wq, Wk, Wv, Wo):
    total, _ = _run(dict(x=x, past_key=past_key, past_value=past_value,
                         Wq=Wq, Wk=Wk, Wv=Wv, Wo=Wo))
    return total
